# revision 9
# baseline (speedup 1.0000x reference)
"""Bass/Tile attention kernel for Trainium2, SPMD over 8 NeuronCores.

Problem: B,S,D,DK = 8,2048,512,64 full bidirectional attention with
softmax; returns (attended[B,S,DK], weights[B,S,S]).

Sharding: data-parallel over batch — core b handles batch b. No
collectives needed. W_q/W_k/W_v replicated.

Per-core dataflow (S=2048 seq, D=512 model, DK=64 head):
  x[2048,512] --PE transpose--> xT[512,2048]
  qT/kT[64,2048] = W.T @ xT     (float32r matmuls, full-rate fp32)
  vT[64,2048] --PE transpose--> v[2048,64] (bf16)
  loop over 16 query tiles t:
    S_t[128,2048] = qT_t.T @ kT         (float32r)
    expS_t = exp(0.125*S_t)  (ACT, accum_out -> Z_t row sums, bf16 out)
    P_t = expS_t * (1/Z_t)   (DVE) --> DMA out weights tile (streams early)
    PE-transpose expS_t tiles -> PT buffer [sk,sq] (bf16)
  attT[64,2048] = sum_u v_u.T @ PT_u    (bf16)
  att = attT.T * (1/Z)  --> DMA out
"""

import numpy as np

B, S, D, DK = 8, 2048, 512, 64
P = 128          # partition size
SQT = S // P     # 16 query tiles
C = D // P       # 4 d-chunks
U = S // P       # 16 key tiles
NCH = S // 512   # 4 free-dim 512-chunks

_CACHE = {}


def _build():
    import concourse.bass as bass
    from concourse import bacc, mybir, tile
    from concourse.masks import make_identity

    f32 = mybir.dt.float32
    f32r = mybir.dt.float32r
    bf16 = mybir.dt.bfloat16
    Exp = mybir.ActivationFunctionType.Exp

    nc = bacc.Bacc("TRN2", target_bir_lowering=False, debug=False,
                   num_devices=B)

    x_d = nc.dram_tensor("x", [S, D], f32, kind="ExternalInput").ap()
    wq_d = nc.dram_tensor("wq", [D, DK], f32, kind="ExternalInput").ap()
    wk_d = nc.dram_tensor("wk", [D, DK], f32, kind="ExternalInput").ap()
    wv_d = nc.dram_tensor("wv", [D, DK], f32, kind="ExternalInput").ap()
    att_d = nc.dram_tensor("att", [S, DK], f32, kind="ExternalOutput").ap()
    wts_d = nc.dram_tensor("wts", [S, S], f32, kind="ExternalOutput").ap()

    with tile.TileContext(nc) as tc:
        with (
            tc.tile_pool(name="const", bufs=1) as const,
            tc.tile_pool(name="xf", bufs=3) as xfpool,
            tc.tile_pool(name="expp", bufs=2) as expp,
            tc.tile_pool(name="pout", bufs=3) as pout,
            tc.tile_pool(name="attsb", bufs=2) as attsb,
            tc.tile_pool(name="spsum", bufs=2, space="PSUM") as spsum,
            tc.tile_pool(name="trpsum", bufs=2, space="PSUM") as trpsum,
            tc.tile_pool(name="attpsum", bufs=2, space="PSUM") as attpsum,
        ):
            ident_f = const.tile([P, P], f32)
            make_identity(nc, ident_f)
            ident_b = const.tile([P, P], bf16)
            make_identity(nc, ident_b)

            # --- weights: [512,64] -> [128, c, 64] (f32r for PE) ---
            wstage = xfpool.tile([P, 3, C, DK], f32, tag="wstage")
            for i, w_d in enumerate((wq_d, wk_d, wv_d)):
                nc.sync.dma_start(
                    wstage[:, i], w_d.rearrange("(c p) k -> p c k", p=P))
            w_sb3 = const.tile([P, 3, C, DK], f32r)
            nc.vector.tensor_copy(w_sb3[:], wstage[:])
            wq_sb = w_sb3[:, 0]
            wk_sb = w_sb3[:, 1]
            wv_sb = w_sb3[:, 2]

            # --- x load + transpose -> xT [128, c, 2048] f32r ---
            xT = const.tile([P, C, S], f32r)
            for t in range(SQT):
                xf = xfpool.tile([P, D], f32, tag="xf")
                nc.sync.dma_start(xf[:], x_d[t * P:(t + 1) * P, :])
                trp = trpsum.tile([P, 512], f32, tag="tr")
                for c in range(C):
                    nc.tensor.transpose(
                        trp[:, c * P:(c + 1) * P], xf[:, c * P:(c + 1) * P],
                        ident_f[:])
                nc.vector.tensor_copy(
                    xT[:, :, t * P:(t + 1) * P],
                    trp[:].rearrange("p (c f) -> p c f", c=C))

            # --- qT, kT [64, 2048] f32 ; vT -> v [128, u*64] bf16 ---
            qT = const.tile([DK, S], f32r)
            kT = const.tile([DK, S], f32r)
            vT = const.tile([DK, S], bf16)
            for n in range(NCH):
                sl = slice(n * 512, (n + 1) * 512)
                for (w_sb, dst) in ((wq_sb, qT), (wk_sb, kT), (wv_sb, vT)):
                    mm = spsum.tile([DK, 512], f32, tag="s")
                    for c in range(C):
                        nc.tensor.matmul(
                            mm[:], w_sb[:, c, :], xT[:, c, sl],
                            start=(c == 0), stop=(c == C - 1))
                    nc.vector.tensor_copy(dst[:, sl], mm[:])

            v_sb = const.tile([P, U * DK], bf16)
            for g in range(2):
                trp = trpsum.tile([P, 512], bf16, tag="tr")
                for j in range(8):
                    u = g * 8 + j
                    nc.tensor.transpose(
                        trp[:, j * DK:(j + 1) * DK],
                        vT[:, u * P:(u + 1) * P], ident_b[:DK, :DK])
                nc.vector.tensor_copy(v_sb[:, g * 512:(g + 1) * 512], trp[:])

            # --- persistent buffers for the main loop ---
            ptbuf = const.tile([P, U, S], bf16)      # PT[sk, u, sq]
            recip = const.tile([P, SQT], f32)        # 1/Z per query tile

            # --- main loop over query tiles ---
            for t in range(SQT):
                tsl = slice(t * P, (t + 1) * P)
                expS = expp.tile([P, S], bf16, tag="exp")
                zp = expp.tile([P, 2], f32, tag="zp")
                for h in range(2):
                    sp = spsum.tile([P, 1024], f32, tag="s")
                    for i in range(2):
                        n = h * 2 + i
                        nc.tensor.matmul(
                            sp[:, i * 512:(i + 1) * 512],
                            qT[:, tsl], kT[:, n * 512:(n + 1) * 512],
                            start=True, stop=True)
                    nc.scalar.activation(
                        expS[:, h * 1024:(h + 1) * 1024], sp[:], Exp,
                        scale=0.125, accum_out=zp[:, h:h + 1])
                zs = expp.tile([P, 1], f32, tag="zs")
                nc.vector.tensor_add(zs[:], zp[:, 0:1], zp[:, 1:2])
                nc.vector.reciprocal(recip[:, t:t + 1], zs[:])

                p_t = pout.tile([P, S], f32, tag="p")
                nc.vector.tensor_scalar_mul(p_t[:], expS[:], recip[:, t:t + 1])
                nc.sync.dma_start(wts_d[tsl, :], p_t[:])

                for g in range(4):
                    trp = trpsum.tile([P, 512], bf16, tag="tr")
                    for j in range(4):
                        u = g * 4 + j
                        nc.tensor.transpose(
                            trp[:, j * P:(j + 1) * P],
                            expS[:, u * P:(u + 1) * P], ident_b[:])
                    nc.vector.tensor_copy(
                        ptbuf[:, g * 4:(g + 1) * 4, tsl],
                        trp[:].rearrange("p (j f) -> p j f", j=4))

            # --- PV: attT[64, 2048] = sum_u v_u.T @ PT_u ---
            attT = attsb.tile([DK, S], bf16, tag="attT")
            for n in range(NCH):
                ap = attpsum.tile([DK, 512], f32, tag="attp")
                for u in range(U):
                    nc.tensor.matmul(
                        ap[:], v_sb[:, u * DK:(u + 1) * DK],
                        ptbuf[:, u, n * 512:(n + 1) * 512],
                        start=(u == 0), stop=(u == U - 1))
                nc.vector.tensor_copy(attT[:, n * 512:(n + 1) * 512], ap[:])

            # --- att = attT.T * recip -> out ---
            for g in range(2):
                trp = trpsum.tile([P, 512], bf16, tag="tr")
                for j in range(8):
                    t = g * 8 + j
                    nc.tensor.transpose(
                        trp[:, j * DK:(j + 1) * DK],
                        attT[:, t * P:(t + 1) * P], ident_b[:DK, :DK])
                for j in range(8):
                    t = g * 8 + j
                    a_t = attsb.tile([P, DK], f32, tag="a")
                    nc.vector.tensor_scalar_mul(
                        a_t[:], trp[:, j * DK:(j + 1) * DK],
                        recip[:, t:t + 1])
                    nc.sync.dma_start(att_d[t * P:(t + 1) * P, :], a_t[:])

    nc.compile()
    return nc


def get_nc():
    if "nc" not in _CACHE:
        _CACHE["nc"] = _build()
    return _CACHE["nc"]


def kernel(inputs, W_q, W_k, W_v):
    from concourse.bass_utils import run_bass_kernel_spmd

    nc = get_nc()
    inputs = np.ascontiguousarray(inputs, dtype=np.float32)
    in_maps = [
        {
            "x": inputs[b],
            "wq": np.ascontiguousarray(W_q, dtype=np.float32),
            "wk": np.ascontiguousarray(W_k, dtype=np.float32),
            "wv": np.ascontiguousarray(W_v, dtype=np.float32),
        }
        for b in range(B)
    ]
    res = run_bass_kernel_spmd(nc, in_maps, core_ids=list(range(B)))
    att = np.stack([res.results[b]["att"] for b in range(B)])
    wts = np.stack([res.results[b]["wts"] for b in range(B)])
    return att, wts
